# revision 1
# baseline (speedup 1.0000x reference)
"""AdaptiveBoxBlurNd Trainium2 kernel.

Strategy:
  - Shard channel-wise across the 8 NeuronCores: core i handles channels
    [2i, 2i+2) of all 4 batches -> 8 images of [768, 768] per core.
  - Device (Bass/Tile, SPMD on 8 cores): builds the summed-area table (SAT)
    of the normalized input: W-direction cumulative sum via the DVE
    tensor_tensor_scan, H-direction cumulative sum via TensorEngine
    matmuls with a lower-triangular ones matrix (block-local) plus a
    rank-1 ones-matmul that adds the running previous-block row into the
    same PSUM accumulation group.
  - Host: per-channel mean/std (global reductions), normalization, the
    4-corner bilinear sampling of the SAT and the final blend.
"""
import sys, time
sys.path.insert(0, '/opt/trn_rl_repo')
import numpy as np

import concourse.bass as bass
import concourse.bacc as bacc
import concourse.mybir as mybir
import concourse.tile as tile
from concourse.bass_utils import run_bass_kernel_spmd

dt = mybir.dt
EPS = 1e-5
B, C, H, W = 4, 16, 768, 768
NCORES = 8
CPS = C // NCORES          # channels per core
IMGS = B * CPS             # images per core
BLK = 128
NBLK = H // BLK

_compiled = None
LAST_SPMD_WALL = None


def _build():
    nc = bacc.Bacc("TRN2", target_bir_lowering=False, debug=False,
                   num_devices=NCORES)
    xn_ext = nc.dram_tensor("xn", [IMGS, H, W], dt.float32, kind="ExternalInput")
    tri_ext = nc.dram_tensor("tri", [BLK, BLK], dt.float32, kind="ExternalInput")
    ones_ext = nc.dram_tensor("ones1", [1, BLK], dt.float32, kind="ExternalInput")
    out_ext = nc.dram_tensor("out", [IMGS, H, W], dt.float32, kind="ExternalOutput")

    xv = xn_ext.ap().rearrange("i (n p) w -> i n p w", p=BLK)
    ov = out_ext.ap().rearrange("i (n p) w -> i n p w", p=BLK)
    HALF = W // 2

    from contextlib import ExitStack
    with ExitStack() as ctx:
        tc = ctx.enter_context(tile.TileContext(nc))
        const = ctx.enter_context(tc.tile_pool(name="const", bufs=1))
        pin = ctx.enter_context(tc.tile_pool(name="pin", bufs=4))
        pw = ctx.enter_context(tc.tile_pool(name="pw", bufs=4))
        ps = ctx.enter_context(tc.tile_pool(name="ps", bufs=4))
        pp = ctx.enter_context(tc.tile_pool(name="pp", bufs=4, space="PSUM"))

        tri = const.tile([BLK, BLK], dt.float32)
        nc.sync.dma_start(out=tri[:], in_=tri_ext.ap())
        ones1 = const.tile([1, BLK], dt.float32)
        nc.sync.dma_start(out=ones1[:], in_=ones_ext.ap())
        ones_col = const.tile([BLK, 1], dt.float32)
        nc.vector.memset(ones_col[:], 1.0)
        zrow = const.tile([BLK, W], dt.float32)
        nc.vector.memset(zrow[:], 0.0)
        prun = ctx.enter_context(tc.tile_pool(name="prun", bufs=2))
        ppc = ctx.enter_context(tc.tile_pool(name="ppc", bufs=4, space="PSUM"))

        for img in range(IMGS):
            running = prun.tile([1, W], dt.float32)
            nc.vector.memset(running[:], 0.0)
            for blk in range(NBLK):
                xt = pin.tile([BLK, W], dt.float32)
                nc.sync.dma_start(out=xt[:], in_=xv[img, blk])
                # cumsum along W on the vector engine
                wc = pw.tile([BLK, W], dt.float32)
                nc.vector.tensor_tensor_scan(
                    wc[:], xt[:], zrow[:], 0.0,
                    mybir.AluOpType.add, mybir.AluOpType.add)
                # cumsum along H: triangular matmul + running-row rank-1 add
                sat = ps.tile([BLK, W], dt.float32)
                for half in range(2):
                    acc = pp.tile([BLK, HALF], dt.float32)
                    sl = slice(half * HALF, (half + 1) * HALF)
                    if blk == 0:
                        nc.tensor.matmul(acc[:], tri[:], wc[:, sl],
                                         start=True, stop=True)
                    else:
                        nc.tensor.matmul(acc[:], tri[:], wc[:, sl],
                                         start=True, stop=False)
                        nc.tensor.matmul(acc[:], ones1[:],
                                         running[0:1, sl],
                                         start=False, stop=True)
                    nc.vector.tensor_copy(sat[:, sl], acc[:])
                    # update running row: += column-sums of this block
                    csum = ppc.tile([1, HALF], dt.float32)
                    nc.tensor.matmul(csum[:], ones_col[:], wc[:, sl],
                                     start=True, stop=True)
                    nc.vector.tensor_add(running[0:1, sl], running[0:1, sl],
                                         csum[:])
                nc.sync.dma_start(out=ov[img, blk], in_=sat[:])
    nc.compile()
    return nc


def _reflect_np(x, size):
    span = np.float32(size - 1)
    x = np.abs(x)
    extra = np.mod(x, span)
    flips = np.floor(x / span)
    x = np.where(np.mod(flips, 2.0) == 0.0, extra, span - extra)
    return np.clip(x, 0.0, span)


def kernel(x, kernel_sizes):
    global _compiled, LAST_SPMD_WALL
    x = np.asarray(x, dtype=np.float32)
    k = np.asarray(kernel_sizes, dtype=np.float32)

    # --- host: per-channel normalization statistics -----------------------
    xd = x.astype(np.float64)
    mean = xd.mean(axis=(0, 2, 3), keepdims=True)
    var = xd.var(axis=(0, 2, 3), ddof=1, keepdims=True)
    std = np.sqrt(var)
    mean32 = mean.astype(np.float32)
    std32 = std.astype(np.float32)
    xn = ((x - mean32) / (std32 + np.float32(EPS))).astype(np.float32)

    # --- device: summed-area table on 8 NeuronCores (channel-sharded) -----
    if _compiled is None:
        _compiled = _build()
    nc = _compiled
    tri_np = np.tril(np.ones((BLK, BLK), dtype=np.float32)).T.copy()
    # lhsT layout: matmul computes lhsT.T @ rhs; we want L (lower tri of ones)
    # as the effective left matrix, so pass L^T.
    ones_np = np.ones((1, BLK), dtype=np.float32)
    in_maps = []
    for core in range(NCORES):
        sh = xn[:, core * CPS:(core + 1) * CPS]          # [B, CPS, H, W]
        in_maps.append({
            "xn": np.ascontiguousarray(sh.reshape(IMGS, H, W)),
            "tri": tri_np,
            "ones1": ones_np,
        })
    t0 = time.time()
    res = run_bass_kernel_spmd(nc, in_maps, core_ids=list(range(NCORES)))
    LAST_SPMD_WALL = time.time() - t0
    sat = np.empty((B, C, H, W), dtype=np.float32)
    for core in range(NCORES):
        sat[:, core * CPS:(core + 1) * CPS] = \
            res.results[core]["out"].reshape(B, CPS, H, W)

    # --- host: 4-corner bilinear sampling of the SAT + blend --------------
    w_idx = np.arange(W, dtype=np.float32)
    h_idx = np.arange(H, dtype=np.float32)
    gx = (-1.0 + 2.0 * w_idx / (W - 1) - 1.0 / W)[None, None, :]   # [1,1,W]
    gy = (-1.0 + 2.0 * h_idx / (H - 1) - 1.0 / H)[None, :, None]   # [1,H,1]
    fx = k[..., 0] / np.float32(W)                                  # [B,H,W]
    fy = k[..., 1] / np.float32(H)
    s = np.zeros((B, C, H, W), dtype=np.float32)
    bidx = np.arange(B)[:, None, None]
    for cx, cy, sign in ((-1., -1., 1.), (-1., 1., -1.), (1., -1., -1.), (1., 1., 1.)):
        ix = _reflect_np((gx + cx * fx + 1.0) * 0.5 * (W - 1), W)
        iy = _reflect_np((gy + cy * fy + 1.0) * 0.5 * (H - 1), H)
        x0 = np.floor(ix)
        y0 = np.floor(iy)
        wx = (ix - x0)[:, None]
        wy = (iy - y0)[:, None]
        x0i = np.clip(x0.astype(np.int32), 0, W - 1)
        x1i = np.clip(x0i + 1, 0, W - 1)
        y0i = np.clip(y0.astype(np.int32), 0, H - 1)
        y1i = np.clip(y0i + 1, 0, H - 1)
        g = lambda yi, xi: sat[bidx[..., None], np.arange(C)[None, :, None, None],
                               yi[:, None], xi[:, None]]
        top = g(y0i, x0i) * (1 - wx) + g(y0i, x1i) * wx
        bot = g(y1i, x0i) * (1 - wx) + g(y1i, x1i) * wx
        s += np.float32(sign) * (top * (1 - wy) + bot * wy)
    areas = (k[..., 0] * k[..., 1])[:, None]
    out = s / (areas + np.float32(EPS)) * std32 + mean32
    return out.astype(np.float32)



# revision 11
# speedup vs baseline: 1.3176x; 1.3176x over previous
"""AdaptiveBoxBlurNd Trainium2 kernel — full on-device pipeline.

Sharding: 8 cores = (batch b, channel-half j); core 2b+j handles channels
[8j, 8j+8) of batch b (full 768x768 spatial extent, so the SAT needs no
cross-core exchange).

Device pipeline per core:
  Phase 1  normalize (per-channel affine from host mean/std) + summed-area
           table: DVE tensor_tensor_scan along W, PE triangular matmul along
           H with a running-row carry. SAT stored to DRAM f32, padded 18
           rows top/bottom.
  Phase 2a per-pixel sampling tables: the 4 corner coordinates with
           reflection, floor -> 16 bilinear tap indices (relative to a
           36-row window) + 16 signed weights + 1/(area+eps), written to
           DRAM in gather-ready layouts.
  Phase 2b per 8-row block: load the 36-row SAT windows (one 16-partition
           group per output row, 16 channels on partitions), GPSIMD
           ap_gather of 16 taps/pixel, weight-multiply (weights replicated
           across the 16 channel partitions with a selector matmul through
           PSUM), tap-reduce, *inv_area, *std + mean, fp16 out.

I/O is fp16/uint16 to halve host<->device transfer: x fp16 in, kernel_sizes
as uint16 fixed-point (k*1024), output fp16.
"""
import sys, time
sys.path.insert(0, '/opt/trn_rl_repo')
import numpy as np

import concourse.bass as bass
import concourse.bacc as bacc
import concourse.mybir as mybir
import concourse.tile as tile
from concourse.bass_utils import run_bass_kernel_spmd

dt = mybir.dt
ALU = mybir.AluOpType
ACTF = mybir.ActivationFunctionType
EPS = 1e-5
B, C, H, W = 4, 16, 768, 768
NCORES = 8
CPS = 8                    # channels per core
BLK = 128                  # phase-1 / plane row block
NBLK = H // BLK
WROWS = 36                 # gather window rows
NELEMS = WROWS * W         # 27648 gather source elems per partition
PAD = 18                   # SAT DRAM pad rows (top/bottom)
NB8 = H // 8               # 96 8-row gather blocks
HALF = W // 2
FLOW = 767.0 / 1536.0      # k -> half-extent coordinate scale
KSCALE = 1.0 / 1024.0      # uint16 fixed-point -> k

_compiled = None
LAST_SPMD_WALL = None
DEBUG_ARTS = False
LAST_RES = None


def _build():
    nc = bacc.Bacc("TRN2", target_bir_lowering=False, debug=False,
                   num_devices=NCORES)

    x16 = nc.dram_tensor("x16", [CPS, H, W], dt.float16, kind="ExternalInput")
    kxu = nc.dram_tensor("kxu", [H, W], dt.uint16, kind="ExternalInput")
    kyu = nc.dram_tensor("kyu", [H, W], dt.uint16, kind="ExternalInput")
    ab = nc.dram_tensor("ab", [128, 16], dt.float32, kind="ExternalInput")
    nrm = nc.dram_tensor("nrm", [128, 2], dt.float32, kind="ExternalInput")
    out16 = nc.dram_tensor("out16", [CPS, H, W], dt.float16,
                           kind="ExternalOutput")

    scratch_kind = "ExternalOutput" if DEBUG_ARTS else "Internal"
    satd = nc.dram_tensor("satd", [CPS, H + 2 * PAD, W], dt.float32,
                          kind=scratch_kind)
    idxt = nc.dram_tensor("idxt", [NB8, 128, W], dt.int16, kind=scratch_kind)
    wti = nc.dram_tensor("wti", [H, 16 * W + W], dt.float32,
                         kind=scratch_kind)

    # --- inline constants -------------------------------------------------
    tri_c = nc.inline_tensor(
        np.tril(np.ones((BLK, BLK), np.float32)).T.copy(), name="tri_c")
    sel_np = np.zeros((8, 128), np.float32)
    for g in range(8):
        sel_np[g, g * 16:(g + 1) * 16] = 1.0
    sel_c = nc.inline_tensor(sel_np, name="sel_c")
    bx_c = nc.inline_tensor(
        (np.arange(W, dtype=np.float64) - 767.0 / 1536.0)
        .astype(np.float32)[None, :], name="bx_c")
    by_np = np.zeros((128, NBLK), np.float32)
    r768_np = np.zeros((128, NBLK), np.float32)
    for blk in range(NBLK):
        r = 128 * blk + np.arange(128, dtype=np.float64)
        by_np[:, blk] = (r - 767.0 / 1536.0).astype(np.float32)
        r768_np[:, blk] = ((17.0 - r) * 768.0).astype(np.float32)
    by_c = nc.inline_tensor(by_np, name="by_c")
    r768_c = nc.inline_tensor(r768_np, name="r768_c")

    satv = satd.ap()
    xv = x16.ap()
    ov = out16.ap()

    from contextlib import ExitStack
    with ExitStack() as octx:
        tc = octx.enter_context(tile.TileContext(nc))
        const = octx.enter_context(tc.tile_pool(name="const", bufs=1))

        tri = const.tile([BLK, BLK], dt.float32)
        nc.sync.dma_start(out=tri[:], in_=tri_c.ap())
        sel8 = const.tile([8, 128], dt.float32)
        nc.sync.dma_start(out=sel8[:], in_=sel_c.ap())
        bxr = const.tile([1, W], dt.float32)
        nc.sync.dma_start(out=bxr[:], in_=bx_c.ap())
        byt = const.tile([128, NBLK], dt.float32)
        nc.sync.dma_start(out=byt[:], in_=by_c.ap())
        r768t = const.tile([128, NBLK], dt.float32)
        nc.sync.dma_start(out=r768t[:], in_=r768_c.ap())
        abt = const.tile([128, 16], dt.float32)
        nc.sync.dma_start(out=abt[:], in_=ab.ap())
        nrmt = const.tile([128, 2], dt.float32)
        nc.sync.dma_start(out=nrmt[:], in_=nrm.ap())
        ones1 = const.tile([1, BLK], dt.float32)
        nc.vector.memset(ones1[:], 1.0)
        ones_col = const.tile([BLK, 1], dt.float32)
        nc.vector.memset(ones_col[:], 1.0)
        zrow = const.tile([BLK, W], dt.float32)
        nc.vector.memset(zrow[:], 0.0)
        # broadcast bx row to all 128 partitions via ones-matmul
        bxb = const.tile([128, W], dt.float32)
        with tc.tile_pool(name="pbx", bufs=2, space="PSUM") as pbx:
            for half in range(2):
                sl = slice(half * HALF, (half + 1) * HALF)
                pb = pbx.tile([128, HALF], dt.float32)
                nc.tensor.matmul(pb[:], ones1[:], bxr[0:1, sl],
                                 start=True, stop=True)
                nc.vector.tensor_copy(bxb[:, sl], pb[:])

        # ================= Phase 1: normalize + SAT ======================
        with tc.tile_pool(name="p1in", bufs=4) as p1in, \
             tc.tile_pool(name="p1n", bufs=4) as p1n, \
             tc.tile_pool(name="p1w", bufs=4) as p1w, \
             tc.tile_pool(name="p1s", bufs=4) as p1s, \
             tc.tile_pool(name="p1r", bufs=2) as p1r, \
             tc.tile_pool(name="p1p", bufs=4, space="PSUM") as p1p, \
             tc.tile_pool(name="p1pc", bufs=4, space="PSUM") as p1pc:
            for ch in range(CPS):
                # zero pads
                nc.sync.dma_start(out=satv[ch, 0:PAD, :], in_=zrow[0:PAD, :])
                nc.sync.dma_start(out=satv[ch, H + PAD:H + 2 * PAD, :],
                                  in_=zrow[0:PAD, :])
                running = p1r.tile([1, W], dt.float32)
                nc.vector.memset(running[:], 0.0)
                for blk in range(NBLK):
                    xt = p1in.tile([BLK, W], dt.float16)
                    nc.sync.dma_start(out=xt[:],
                                      in_=xv[ch, blk * BLK:(blk + 1) * BLK, :])
                    xn = p1n.tile([BLK, W], dt.float32)
                    nc.vector.tensor_scalar(xn[:], xt[:],
                                            abt[:, ch:ch + 1],
                                            abt[:, 8 + ch:9 + ch],
                                            ALU.mult, ALU.add)
                    wc = p1w.tile([BLK, W], dt.float32)
                    nc.vector.tensor_tensor_scan(wc[:], xn[:], zrow[:], 0.0,
                                                 ALU.add, ALU.add)
                    sats = p1s.tile([BLK, W], dt.float32)
                    for half in range(2):
                        sl = slice(half * HALF, (half + 1) * HALF)
                        acc = p1p.tile([BLK, HALF], dt.float32)
                        if blk == 0:
                            nc.tensor.matmul(acc[:], tri[:], wc[:, sl],
                                             start=True, stop=True)
                        else:
                            nc.tensor.matmul(acc[:], tri[:], wc[:, sl],
                                             start=True, stop=False)
                            nc.tensor.matmul(acc[:], ones1[:],
                                             running[0:1, sl],
                                             start=False, stop=True)
                        nc.vector.tensor_copy(sats[:, sl], acc[:])
                        csum = p1pc.tile([1, HALF], dt.float32)
                        nc.tensor.matmul(csum[:], ones_col[:], wc[:, sl],
                                         start=True, stop=True)
                        nc.vector.tensor_add(running[0:1, sl],
                                             running[0:1, sl], csum[:])
                    nc.sync.dma_start(
                        out=satv[ch, PAD + blk * BLK:PAD + (blk + 1) * BLK, :],
                        in_=sats[:])

        tc.strict_bb_all_engine_barrier()

        # ================= Phase 2a: sampling tables =====================
        def reflect_floor(pool, coord, tag):
            """coord [128, W] f32 -> (x0f, wx, x1f) after reflection.

            Uses 5 working tiles (a, b, c, d, i32); returns b=x0f, d=wx,
            a=x1f which stay live after the call.
            """
            a = pool.tile([128, W], dt.float32, name=f"a_{tag}")
            nc.scalar.activation(a[:], coord[:], ACTF.Abs)       # a = |ix|
            b = pool.tile([128, W], dt.float32, name=f"b_{tag}")
            nc.vector.tensor_scalar(b[:], a[:], 1.0 / 767.0, -0.5,
                                    ALU.mult, ALU.add)
            i32 = pool.tile([128, W], dt.int32, name=f"i_{tag}")
            nc.vector.tensor_copy(i32[:], b[:])
            nc.vector.tensor_copy(b[:], i32[:])                  # b = flips
            c = pool.tile([128, W], dt.float32, name=f"c_{tag}")
            nc.vector.tensor_scalar(c[:], b[:], 767.0, None, ALU.mult)
            nc.vector.tensor_tensor(c[:], a[:], c[:], ALU.subtract)  # extra
            d = pool.tile([128, W], dt.float32, name=f"d_{tag}")
            nc.vector.tensor_scalar(d[:], c[:], -2.0, 767.0,
                                    ALU.mult, ALU.add)           # 767-2ex
            nc.vector.tensor_tensor(d[:], b[:], d[:], ALU.mult)
            nc.vector.tensor_tensor(c[:], c[:], d[:], ALU.add)   # c = refl
            nc.vector.tensor_scalar(c[:], c[:], 0.0, 767.0,
                                    ALU.max, ALU.min)
            nc.vector.tensor_scalar(d[:], c[:], 0.5, None, ALU.subtract)
            nc.vector.tensor_copy(i32[:], d[:])
            nc.vector.tensor_copy(b[:], i32[:])                  # b = x0f
            nc.vector.tensor_tensor(d[:], c[:], b[:], ALU.subtract)  # d = wx
            nc.vector.tensor_scalar(a[:], b[:], 1.0, 767.0,
                                    ALU.add, ALU.min)            # a = x1f
            return b, d, a

        idv = idxt.ap()
        wtv = wti.ap()
        with tc.tile_pool(name="p2k", bufs=2) as p2k, \
             tc.tile_pool(name="p2a", bufs=1) as p2a, \
             tc.tile_pool(name="p2w", bufs=1) as p2w:
            for pb in range(NBLK):
                rsl = slice(pb * BLK, (pb + 1) * BLK)
                kx = p2k.tile([128, W], dt.uint16, name="kx")
                nc.sync.dma_start(out=kx[:], in_=kxu.ap()[rsl, :])
                ky = p2k.tile([128, W], dt.uint16, name="ky")
                nc.sync.dma_start(out=ky[:], in_=kyu.ap()[rsl, :])
                kxf = p2a.tile([128, W], dt.float32)
                nc.vector.tensor_scalar(kxf[:], kx[:], KSCALE, None, ALU.mult)
                kyf = p2a.tile([128, W], dt.float32)
                nc.vector.tensor_scalar(kyf[:], ky[:], KSCALE, None, ALU.mult)

                w16 = p2w.tile([128, 17 * W], dt.float32)
                # inv area into cols [16W, 17W)
                ar = p2a.tile([128, W], dt.float32)
                nc.vector.tensor_tensor(ar[:], kxf[:], kyf[:], ALU.mult)
                nc.vector.tensor_scalar(ar[:], ar[:], EPS, None, ALU.add)
                nc.vector.reciprocal(w16[:, 16 * W:17 * W], ar[:])

                fx = p2a.tile([128, W], dt.float32)
                nc.vector.tensor_scalar(fx[:], kxf[:], FLOW, None, ALU.mult)
                fy = p2a.tile([128, W], dt.float32)
                nc.vector.tensor_scalar(fy[:], kyf[:], FLOW, None, ALU.mult)

                xs = []     # (x0f/x1f planes), 4 entries l0,l1,r0,r1
                wxs = []    # signed weight planes
                for ci, cx in enumerate((-1.0, 1.0)):
                    ix = p2a.tile([128, W], dt.float32, name=f"ix_{ci}")
                    nc.vector.tensor_tensor(
                        ix[:], bxb[:], fx[:],
                        ALU.subtract if cx < 0 else ALU.add)
                    x0f, wx, x1f = reflect_floor(p2a, ix, f"x{ci}")
                    w0 = p2a.tile([128, W], dt.float32, name=f"w0_x{ci}")
                    if cx < 0:
                        nc.vector.tensor_scalar(w0[:], wx[:], 1.0, None,
                                                ALU.subtract)
                        nc.vector.tensor_scalar(wx[:], wx[:], -1.0, None,
                                                ALU.mult)
                    else:
                        nc.vector.tensor_scalar(w0[:], wx[:], -1.0, 1.0,
                                                ALU.mult, ALU.add)
                    xs += [x0f, x1f]
                    wxs += [w0, wx]

                dys = []
                wys = []
                for ci, cy in enumerate((-1.0, 1.0)):
                    iy = p2a.tile([128, W], dt.float32, name=f"iy_{ci}")
                    nc.vector.tensor_scalar(iy[:], fy[:],
                                            -1.0 if cy < 0 else 1.0,
                                            byt[:, pb:pb + 1],
                                            ALU.mult, ALU.add)
                    y0f, wy, y1f = reflect_floor(p2a, iy, f"y{ci}")
                    # dy planes in-place: (y - r + 17)*768
                    nc.vector.tensor_scalar(y0f[:], y0f[:], 768.0,
                                            r768t[:, pb:pb + 1],
                                            ALU.mult, ALU.add)
                    nc.vector.tensor_scalar(y1f[:], y1f[:], 768.0,
                                            r768t[:, pb:pb + 1],
                                            ALU.mult, ALU.add)
                    v0 = p2a.tile([128, W], dt.float32, name=f"v0_y{ci}")
                    if cy < 0:
                        nc.vector.tensor_scalar(v0[:], wy[:], 1.0, None,
                                                ALU.subtract)
                        nc.vector.tensor_scalar(wy[:], wy[:], -1.0, None,
                                                ALU.mult)
                    else:
                        nc.vector.tensor_scalar(v0[:], wy[:], -1.0, 1.0,
                                                ALU.mult, ALU.add)
                    dys += [y0f, y1f]
                    wys += [v0, wy]

                w16v = w16[:, 0:16 * W].rearrange("p (w t) -> p w t", t=16)
                for yi in range(4):
                    for xi in range(4):
                        t_ = 4 * yi + xi
                        idxf = p2a.tile([128, W], dt.float32, name="idxf")
                        nc.vector.tensor_tensor(idxf[:], dys[yi][:],
                                                xs[xi][:], ALU.add)
                        idx6 = p2a.tile([128, W], dt.int16, name="idx6")
                        nc.vector.tensor_copy(idx6[:], idxf[:])
                        # write idxt[b8, g*16+t, :] for rows r=8*b8+g
                        dst = idv[16 * pb:16 * (pb + 1), :, :] \
                            .rearrange("b (g t) w -> b g t w", g=8)[:, :, t_, :]
                        nc.sync.dma_start(out=dst, in_=idx6[:])
                        nc.vector.tensor_tensor(w16v[:, :, t_], wys[yi][:],
                                                wxs[xi][:], ALU.mult)
                nc.sync.dma_start(out=wtv[rsl, :], in_=w16[:])

        tc.strict_bb_all_engine_barrier()

        # ================= Phase 2b: gather + blend ======================
        with tc.tile_pool(name="pwin", bufs=1) as pwin, \
             tc.tile_pool(name="pgth", bufs=1) as pgth, \
             tc.tile_pool(name="pidx", bufs=2) as pidx, \
             tc.tile_pool(name="pwr", bufs=1) as pwr, \
             tc.tile_pool(name="par", bufs=2) as par, \
             tc.tile_pool(name="pia", bufs=2) as pia, \
             tc.tile_pool(name="ps", bufs=2) as ps, \
             tc.tile_pool(name="po", bufs=2) as po, \
             tc.tile_pool(name="ppw", bufs=4, space="PSUM") as ppw, \
             tc.tile_pool(name="ppa", bufs=2, space="PSUM") as ppa:
            win = pwin.tile([128, NELEMS], dt.float32)
            nc.vector.memset(win[:], 0.0)
            for b8 in range(NB8):
                for g in range(8):
                    r = 8 * b8 + g
                    dstw = win[:].rearrange("(a b) f -> a b f", a=8)[g]
                    nc.sync.dma_start(out=dstw[0:CPS, :],
                                      in_=satv[:, r + 1:r + 1 + WROWS, :])
                idx = pidx.tile([128, W], dt.int16)
                nc.sync.dma_start(out=idx[:], in_=idv[b8])
                arr = par.tile([8, W], dt.float32)
                nc.sync.dma_start(out=arr[:],
                                  in_=wtv[8 * b8:8 * b8 + 8, 16 * W:17 * W])
                ia = pia.tile([128, W], dt.float32)
                for hq in range(2):
                    sl = slice(hq * HALF, (hq + 1) * HALF)
                    pa = ppa.tile([128, HALF], dt.float32)
                    nc.tensor.matmul(pa[:], sel8[:], arr[:, sl],
                                     start=True, stop=True)
                    nc.vector.tensor_copy(ia[:, sl], pa[:])
                outt = po.tile([128, W], dt.float16)
                for hf in range(2):
                    wr = pwr.tile([8, 16 * HALF], dt.float32, name="wr")
                    nc.sync.dma_start(
                        out=wr[:],
                        in_=wtv[8 * b8:8 * b8 + 8,
                                hf * 16 * HALF:(hf + 1) * 16 * HALF])
                    gth = pgth.tile([128, 16 * HALF], dt.float32)
                    nc.gpsimd.ap_gather(gth[:], win[:],
                                        idx[:, hf * HALF:(hf + 1) * HALF],
                                        channels=128, num_elems=NELEMS,
                                        d=1, num_idxs=16 * HALF)
                    for cc in range(12):
                        cs = slice(cc * 512, (cc + 1) * 512)
                        pw = ppw.tile([128, 512], dt.float32)
                        nc.tensor.matmul(pw[:], sel8[:], wr[:, cs],
                                         start=True, stop=True)
                        nc.vector.tensor_tensor(gth[:, cs], gth[:, cs],
                                                pw[:], ALU.mult)
                    s = ps.tile([128, HALF], dt.float32)
                    nc.vector.tensor_reduce(
                        s[:], gth[:].rearrange("p (w t) -> p w t", t=16),
                        mybir.AxisListType.X, ALU.add)
                    hsl = slice(hf * HALF, (hf + 1) * HALF)
                    nc.vector.tensor_tensor(s[:], s[:], ia[:, hsl], ALU.mult)
                    nc.vector.tensor_scalar(outt[:, hsl], s[:],
                                            nrmt[:, 0:1], nrmt[:, 1:2],
                                            ALU.mult, ALU.add)
                ovv = outt[:].rearrange("(g c) w -> g c w", c=16)
                for ch in range(CPS):
                    nc.sync.dma_start(out=ov[ch, 8 * b8:8 * b8 + 8, :],
                                      in_=ovv[:, ch, :])
    nc.compile()
    return nc


def kernel(x, kernel_sizes):
    global _compiled, LAST_SPMD_WALL
    x = np.asarray(x, dtype=np.float32)
    k = np.asarray(kernel_sizes, dtype=np.float32)

    # host: per-channel stats (the only cross-element reduction)
    mean = np.mean(x, axis=(0, 2, 3), dtype=np.float64)
    var = np.var(x, axis=(0, 2, 3), ddof=1, dtype=np.float64)
    std = np.sqrt(var).astype(np.float32)
    mean = mean.astype(np.float32)
    istd = 1.0 / (std + np.float32(EPS))

    if _compiled is None:
        _compiled = _build()
    nc = _compiled

    ku = np.round(k * 1024.0).astype(np.uint16)   # [B, H, W, 2]
    in_maps = []
    for core in range(NCORES):
        b, j = divmod(core, 2)
        chs = slice(8 * j, 8 * j + 8)
        ab = np.zeros((128, 16), np.float32)
        ab[:, 0:8] = istd[chs][None, :]
        ab[:, 8:16] = (-mean[chs] * istd[chs])[None, :]
        nrm = np.zeros((128, 2), np.float32)
        cidx = np.arange(128) % 16 % 8 + 8 * j
        nrm[:, 0] = std[cidx]
        nrm[:, 1] = mean[cidx]
        in_maps.append({
            "x16": np.ascontiguousarray(x[b, chs]).astype(np.float16),
            "kxu": np.ascontiguousarray(ku[b, :, :, 0]),
            "kyu": np.ascontiguousarray(ku[b, :, :, 1]),
            "ab": ab,
            "nrm": nrm,
        })
    t0 = time.time()
    res = run_bass_kernel_spmd(nc, in_maps, core_ids=list(range(NCORES)))
    LAST_SPMD_WALL = time.time() - t0
    global LAST_RES
    LAST_RES = res

    out = np.empty((B, C, H, W), dtype=np.float32)
    for core in range(NCORES):
        b, j = divmod(core, 2)
        out[b, 8 * j:8 * j + 8] = res.results[core]["out16"].astype(np.float32)
    return out
